# revision 108
# baseline (speedup 1.0000x reference)
"""Causal multi-head attention on 8 Trainium2 NeuronCores.

Problem: x[4, 2048, 1024], 16 heads of 64; q/k/v = x@W* + b*, causal
softmax attention, out = y@Wp + bp.

Sharding: core c handles batch b = c//2 and head-group hg = c%2
(8 heads = 512 feature columns of Wq/Wk/Wv, 512 rows of Wp).  Each core
computes a full [2048, 1024] partial of the output projection for its
batch; the host sums the two partials per batch and adds bp.

Per-core dataflow (bf16 matmul inputs, fp32 PSUM accumulation):
  * x arrives in DRAM already TRANSPOSED by the host (xt [D, T]) so xT
    streams in as cheap flat DMAs interleaved with the weights in
    first-use order; wq/wk are m-chunked on the host so each 256KB
    first-use piece is one contiguous transfer.  No device transposes
    of x at all.  bk/bv never reach the device: q.(k+bk) shifts every
    score in a softmax row equally (cancels exactly), and since softmax
    rows sum to 1, bv contributes the constant bv@Wp folded into the
    host-side output bias.
  * a PE p-state warmup (dummy matmuls on memset scratch) runs during
    the cold DMA window so the 3us clock ramp completes before real
    work starts.
  * qT/kT [head-pair-on-partitions, t] and v [t, heads*65] (65th column
    of each head's v block is ones so softmax denominators fall out of
    the same AV matmuls).
  * attention runs in QUARTER-passes of one head pair (hc): per
    (quarter, k-block 128, q-super-block 512) BOTH heads' score tiles
    land in one [128, 2, 512] two-bank PSUM tile and share ONE exp
    instruction (3D AP; halves the ACT per-instruction overhead, the
    second engine bottleneck).  Triangular mask on the diagonal blocks
    is one stride-0-broadcast DVE multiply for both heads.
  * AV is REORIENTED: per q-chunk of 128, y[q, 65] += sT_chunk.T @ v_ext
    (lhsT = sT chunk) streams only 65 output rows per (q128, k128) pair
    instead of 512 — half the PE time of the yT-oriented form.  The 4
    slots (2 qc x 2 heads) of a PSUM y-bank share ONE accumulation
    group (hardware zero-regions are whole banks), opened by the bank's
    first AV and closed by its last, so early q-chunks normalize while
    later k-blocks still accumulate the other bank.
  * softmax normalization is a per-partition scalar multiply (DVE) while
    copying y PSUM->SBUF bf16; y is DMA-transposed to yT for the output
    projection, whose result is staged bf16 and DMA'd to DRAM (the host
    sums the two partials per batch in fp32).  For the FINAL jq the yT
    transpose runs on the PE instead (it is starving there anyway),
    skipping the ~3.7us XBAR DMA chain on the critical tail, and each
    nd half DMAs on its own so the last transfer is small; earlier jqs
    fuse both nd halves into one whole-row DMA (half the HWDGE issues).
  * the PE instruction stream is software-pipelined by an emission
    scheduler: projection / output-projection matmuls carry deadline
    tags (first-use attention step) and are paced into the exp-limited
    attention steps by a most-binding-prefix rate, with out-projection
    work deferred into the late, exp-bound region.  Work-group PSUM
    tiles alternate across two pools so group copies never serialize.
  * within the ACT-bound final jq, pending AVs carry ACROSS quarter
    boundaries (qt<3 norms queue no outproj work so deferring them is
    free; ybanks allocate lazily at each quarter's first AV) — the exp
    chain never stalls waiting for an end-of-quarter drain.
"""
import numpy as np
from collections import deque

B, T, D = 4, 2048, 1024
NH, HD = 16, 64
NHL = 8            # heads per core
DL = NHL * HD      # 512: local qkv feature width
P = 128
QB = 512           # q super-block (columns of sT tiles)
NQ = T // QB       # 4
NKT = T // P       # 16 k blocks
KC = D // P        # 8 contraction chunks over model dim
FC = DL // P       # 4 chunks over local feature dim
DB = 512           # out-projection column block
ND = D // DB       # 2
VE = HD + 1        # 65: v block width incl ones column
LAG = 7            # exp->AV software pipeline depth (in (ik,h) steps)
LAGF = 8           # pipeline depth for the final jq (8 = same as LAG)
OPTH = 28          # outproj pacing horizon (steps past TOT)
BNDI = 3           # base index where the late feed region starts
LATE = 1.212       # outproj feed factor in the exp-bound late region
EARLY = 0.3        # outproj feed factor before the late region
FQB = 0.0          # extra per-step outproj top-up in the final quarter
FPC = 7            # v-proj contraction chunk that runs fp8-DoubleRow

_CACHE = {}


def _build():
    import concourse.bass as bass
    from concourse import bacc
    import concourse.mybir as mybir
    import concourse.tile as tile

    f32 = mybir.dt.float32
    bf16 = mybir.dt.bfloat16
    Exp = mybir.ActivationFunctionType.Exp

    f8 = mybir.dt.float8e4
    DR = mybir.MatmulPerfMode.DoubleRow

    nc = bacc.Bacc(None)
    xt_d = nc.dram_tensor("xt", [D, T], bf16, kind="ExternalInput")
    # ONE contraction chunk (FPC) of the v projection runs fp8-DoubleRow
    # (half-rate rows): costs 1.4e-2 rel err vs the 2e-2 gate. Operands
    # arrive host-packed [K/2, 2, N] with k = i*64 + p on both sides.
    xt8_d = nc.dram_tensor("xt8", [P // 2, 2, T], f8, kind="ExternalInput")
    wv8_d = nc.dram_tensor("wv8", [P // 2, 2, DL], f8, kind="ExternalInput")
    # wq/wk arrive m-chunked from the host ([m, p, c, w] with w the
    # 128-col lhsT slice) so each 256KB chunk is one contiguous DMA
    wq_d = nc.dram_tensor("wq", [FC, P, KC, P], bf16, kind="ExternalInput")
    wk_d = nc.dram_tensor("wk", [FC, P, KC, P], bf16, kind="ExternalInput")
    wv_d = nc.dram_tensor("wv", [D, DL], bf16, kind="ExternalInput")
    wp_d = nc.dram_tensor("wp", [DL, D], bf16, kind="ExternalInput")
    bq_d = nc.dram_tensor("bq", [DL], f32, kind="ExternalInput")
    mask_d = nc.dram_tensor("mask", [P, P], bf16, kind="ExternalInput")
    ident_d = nc.dram_tensor("ident", [P, P], bf16, kind="ExternalInput")
    out_d = nc.dram_tensor("out", [T, D], bf16, kind="ExternalOutput")
    # bk and bv never reach the device: q.(k+bk) shifts every score in a
    # softmax row by the same q.bk, so bk cancels exactly; and since
    # softmax rows sum to 1, v's bias adds a constant bv to y, so
    # bv@Wp folds into the host-side output bias.

    with tile.TileContext(nc) as tc:
        with (
            tc.tile_pool(name="const", bufs=1) as const,
            tc.tile_pool(name="big", bufs=1) as big,
            tc.tile_pool(name="sT", bufs=23) as sT_pool,
            tc.tile_pool(name="ysb", bufs=3) as ysb_pool,
            tc.tile_pool(name="rec", bufs=4) as rec_pool,
            tc.tile_pool(name="ostage", bufs=6) as stage_pool,
            tc.tile_pool(name="ps_s", bufs=2, space="PSUM") as ps_s,
            tc.tile_pool(name="ps_y", bufs=3, space="PSUM") as ps_y,
            tc.tile_pool(name="ps_w", bufs=1, space="PSUM") as ps_w,
        ):
            # ---- persistent SBUF ----
            xT_sb = big.tile([P, KC, T], bf16)
            wq_sb = big.tile([P, FC, KC, P], bf16)
            wk_sb = big.tile([P, FC, KC, P], bf16)
            wv_sb = big.tile([P, KC, DL], bf16)
            wp_sb = big.tile([P, FC, D], bf16)
            qT_sb = big.tile([P, FC, T], bf16)
            kT_sb = big.tile([P, FC, T], bf16)
            v_sb = big.tile([P, NKT, NHL, VE], bf16)
            yT_sb = big.tile([P, FC, T], bf16)
            xt8_sb = big.tile([P // 2, 2, T], f8)
            wv8_sb = big.tile([P // 2, 2, DL], f8)
            mask_sb = const.tile([P, P], bf16)
            ident_sb = const.tile([P, P], bf16)
            bq_sb = const.tile([P, FC], f32)

            # ---- prologue DMAs (SP HWDGE queue; order = issue order,
            # sequenced to match first-use times of the PE stream).  x
            # arrives already transposed from the host, so xT streams in
            # as flat DMAs interleaved with the weights by first use.
            # wq/wk are m-chunked on the host so each first-use piece is
            # a short contiguous 256KB transfer. ----
            xt_r = xt_d.ap().rearrange("(c p) t -> p c t", p=P)
            H = 2 * P   # wv half-width: 256 cols = 512B rows, full DMA bw
            nc.sync.dma_start(wk_sb[:, 0], wk_d.ap()[0])
            for cp in range(2):
                nc.sync.dma_start(
                    xT_sb[:, 2 * cp:2 * cp + 2, 0:QB],
                    xt_r[:, 2 * cp:2 * cp + 2, 0:QB])
            nc.sync.dma_start(wk_sb[:, 1], wk_d.ap()[1])
            nc.sync.dma_start(xT_sb[:, 4:8, 0:QB], xt_r[:, 4:8, 0:QB])
            nc.sync.dma_start(wq_sb[:, 0], wq_d.ap()[0])
            wv_r = wv_d.ap().rearrange("(c p) m -> p c m", p=P)
            nc.sync.dma_start(wv_sb[:, :, 0:H], wv_r[:, :, 0:H])
            nc.sync.dma_start(wv8_sb, wv8_d.ap())
            nc.sync.dma_start(xt8_sb, xt8_d.ap())
            nc.sync.dma_start(bq_sb, bq_d.ap().rearrange("(c p) -> p c", p=P))
            nc.sync.dma_start(mask_sb, mask_d.ap())
            nc.sync.dma_start(wq_sb[:, 1], wq_d.ap()[1])
            nc.sync.dma_start(wk_sb[:, 2], wk_d.ap()[2])
            nc.sync.dma_start(wq_sb[:, 2], wq_d.ap()[2])
            nc.sync.dma_start(wv_sb[:, :, H:DL], wv_r[:, :, H:DL])
            nc.sync.dma_start(wk_sb[:, 3], wk_d.ap()[3])
            nc.sync.dma_start(wq_sb[:, 3], wq_d.ap()[3])
            for seg in range(1, NQ):
                nc.sync.dma_start(
                    xT_sb[:, :, seg * QB:(seg + 1) * QB],
                    xt_r[:, :, seg * QB:(seg + 1) * QB])
            nc.sync.dma_start(
                wp_sb, wp_d.ap().rearrange("(c p) m -> p c m", p=P))
            nc.sync.dma_start(ident_sb, ident_d.ap())

            # ---- PE p-state warmup: dummy matmuls on a memset scratch
            # keep the PE busy from ~1.5us so the 3us continuous-busy
            # ramp completes early; more warmups are interleaved into the
            # first projection group below wherever the cold DMA chain
            # would otherwise stall (and reset the ramp) ----
            wup_sb = const.tile([P, QB], bf16)
            nc.vector.memset(wup_sb, 0.0)
            nc.vector.memset(v_sb[:, :, :, HD], 1.0)   # ones columns
            wups = ps_w.tile([P, QB], f32, name="psw")

            def wup(n):
                for r in range(n):
                    nc.tensor.matmul(wups, lhsT=wup_sb[:, 0:P],
                                     rhs=wup_sb, start=True, stop=True)
            wup(7)

            # ---- work-item machinery (each item emits ~1 PE matmul) ----
            def group_items(n_mm, emit_mm, emit_tail):
                cell = {}
                items = []
                for i in range(n_mm):
                    def it(i=i):
                        emit_mm(i, cell)
                        if i == n_mm - 1:
                            emit_tail(cell)
                    items.append(it)
                return items

            # work-group PSUM: alternate between the ps_w bank and the
            # (mostly idle between y-uses) ps_y banks so pipelined groups
            # never serialize on one bank's copy latency
            rot = [0]

            def mkps_rot():
                rot[0] ^= 1
                return (ps_w.tile([P, QB], f32, name="psw") if rot[0]
                        else ps_y.tile([P, QB], f32, name="yb"))

            def v_group(jt, hv):
                """Half of the v projection for t-block jt: feature columns
                [hv*256, (hv+1)*256) = heads 4hv..4hv+3."""
                c0, c1 = hv * H, (hv + 1) * H

                def mm(i, cell):
                    if i == 0:
                        cell["ps"] = mkps_rot()
                    if i == FPC:
                        nc.tensor.matmul(
                            cell["ps"][:, c0:c1],
                            lhsT=xt8_sb[:, :, jt * P:(jt + 1) * P],
                            rhs=wv8_sb[:, :, c0:c1],
                            start=(i == 0), stop=(i == KC - 1),
                            perf_mode=DR)
                    else:
                        nc.tensor.matmul(
                            cell["ps"][:, c0:c1],
                            lhsT=xT_sb[:, i, jt * P:(jt + 1) * P],
                            rhs=wv_sb[:, i, c0:c1],
                            start=(i == 0), stop=(i == KC - 1))

                def tail(cell):
                    nc.vector.tensor_copy(
                        v_sb[:, jt, 4 * hv:4 * hv + 4, 0:HD],
                        cell["ps"][:, c0:c1].rearrange(
                            "p (h e) -> p h e", h=4))
                return group_items(KC, mm, tail)

            def qk_group(w_sb, b_sb, dst, m, seg, mkps=None):
                def mm(i, cell):
                    if i == 0:
                        cell["ps"] = mkps() if mkps else mkps_rot()
                    nc.tensor.matmul(
                        cell["ps"],
                        lhsT=w_sb[:, m, i, :],
                        rhs=xT_sb[:, i, seg * QB:(seg + 1) * QB],
                        start=(i == 0), stop=(i == KC - 1))

                def tail(cell):
                    if b_sb is None:
                        nc.vector.tensor_copy(
                            dst[:, m, seg * QB:(seg + 1) * QB], cell["ps"])
                    else:
                        nc.vector.tensor_scalar_add(
                            dst[:, m, seg * QB:(seg + 1) * QB], cell["ps"],
                            b_sb[:, m:m + 1])
                return group_items(KC, mm, tail)

            def outproj_group(tb, nd, shared, fuse, h=None, mkps=None):
                """fuse: both nd halves land in one shared [P, D] stage
                tile and ONE whole-row DMA fires at nd=1's tail (halves
                the HWDGE issue load); unfused (final jq) each nd half
                DMAs on its own so the last transfer is small; h splits
                an unfused half again (very last group only)."""
                c0 = nd * DB + (0 if h is None else h * (DB // 2))
                w = DB if h is None else DB // 2

                def mm(c, cell):
                    if c == 0:
                        cell["ps"] = (mkps or mkps_rot)()
                    nc.tensor.matmul(
                        cell["ps"][:, 0:w],
                        lhsT=yT_sb[:, c, tb * P:(tb + 1) * P],
                        rhs=wp_sb[:, c, c0:c0 + w],
                        start=(c == 0), stop=(c == FC - 1))

                def tail(cell):
                    if "st" not in shared:
                        shared["st"] = stage_pool.tile(
                            [P, D], bf16, name="ostage")
                    ot = shared["st"]
                    nc.vector.tensor_copy(
                        ot[:, c0:c0 + w], cell["ps"][:, 0:w])
                    if fuse and nd == ND - 1:
                        nc.sync.dma_start(
                            out_d.ap()[tb * P:(tb + 1) * P, :], ot)
                    elif not fuse:
                        nc.sync.dma_start(
                            out_d.ap()[tb * P:(tb + 1) * P, c0:c0 + w],
                            ot[:, c0:c0 + w])
                return group_items(FC, mm, tail)

            def seg_groups(seg):
                gs = []
                gs += qk_group(wk_sb, None, kT_sb, 0, seg)
                gs += qk_group(wk_sb, None, kT_sb, 1, seg)
                gs += qk_group(wq_sb, bq_sb, qT_sb, 0, seg)
                gs += qk_group(wq_sb, bq_sb, qT_sb, 1, seg)
                for jt in range(seg * NQ, seg * NQ + NQ):
                    gs += v_group(jt)
                gs += qk_group(wk_sb, None, kT_sb, 2, seg)
                gs += qk_group(wk_sb, None, kT_sb, 3, seg)
                gs += qk_group(wq_sb, bq_sb, qT_sb, 2, seg)
                gs += qk_group(wq_sb, bq_sb, qT_sb, 3, seg)
                return gs

            # global step index: one step per (jq, quarter=head-pair, ik)
            base = [0]
            for j in range(NQ):
                base.append(base[-1] + 4 * (4 * j + 4))
            TOT = base[NQ]   # 160

            def sidx(jq, qt, ik=0):
                return base[jq] + qt * (4 * jq + 4) + ik

            proj_q = deque()   # items: (due_step, fn) in due order
            opt_q = deque()    # out-projection items (no deadline)
            pace = [0.0, 0.0]

            def seg_items(seg):
                """Projection work for seg, tagged with first-use steps.
                kT/qT chunk m is first read by quarter m (hc == m)."""
                its = []
                for m in range(FC):
                    its += [(sidx(seg, m), f) for f in
                            qk_group(wk_sb, None, kT_sb, m, seg)]
                    its += [(sidx(seg, m), f) for f in
                            qk_group(wq_sb, bq_sb, qT_sb, m, seg)]
                    if m < 2:
                        for jt in (seg * NQ + 2 * m, seg * NQ + 2 * m + 1):
                            for hv in range(2):
                                due = min(sidx(seg, 2 * hv, jt) + LAG + 1,
                                          sidx(seg, 2 * hv + 1) - 1)
                                its += [(due, f) for f in v_group(jt, hv)]
                its.sort(key=lambda t: t[0])
                return its

            def sprinkle(g):
                """Pace proj work by most-binding-prefix rate; spread opt
                work over all remaining steps."""
                if proj_q:
                    r = max((i + 1) / max(1, due - g)
                            for i, (due, _) in enumerate(
                                list(proj_q)[:24]))
                    pace[0] += min(r, 2.0)
                    while pace[0] >= 1.0 and proj_q:
                        proj_q.popleft()[1]()
                        pace[0] -= 1.0
                if opt_q:
                    # defer outproj into the late (exp-bound) region and
                    # hold a reserve for the ACT-bound final quarter
                    r = len(opt_q) / max(1.0, TOT + OPTH - g)
                    f = EARLY if g < base[BNDI] else LATE
                    pace[1] += r * f
                    if g >= base[NQ - 1] + 3 * (4 * (NQ - 1) + 4):
                        # final quarter: steps are exp-clocked ~2 items
                        # slower than PE; rate-pacing underfeeds, so top up
                        pace[1] += FQB
                    while pace[1] >= 1.0 and opt_q:
                        opt_q.popleft()()
                        pace[1] -= 1.0

            # ---- prologue: just enough of seg 0 to start attention;
            # groups run on the idle sT psum banks for deep pipelining ----
            def ps_proto():
                return ps_s.tile([P, 2, QB], f32, name="sT_ps")[:, 0, :]

            # first group: warmup filler between i-chunks where the cold
            # DMA chain (xt45/xt67 still in flight) would stall the PE
            for f in qk_group(wk_sb, None, kT_sb, 0, 0, ps_proto):
                f()
            for f in qk_group(wk_sb, None, kT_sb, 1, 0, ps_proto):
                f()
            for f in qk_group(wq_sb, bq_sb, qT_sb, 0, 0, ps_proto):
                f()
            pro = []
            pro += [(sidx(0, 1), f) for f in
                    qk_group(wq_sb, bq_sb, qT_sb, 1, 0, ps_proto)]
            for m in (2, 3):
                pro += [(sidx(0, m), f) for f in
                        qk_group(wk_sb, None, kT_sb, m, 0, ps_proto)]
                pro += [(sidx(0, m), f) for f in
                        qk_group(wq_sb, bq_sb, qT_sb, m, 0, ps_proto)]
            for jt in range(NQ):
                for hv in range(2):
                    due = min(sidx(0, 2 * hv, jt) + LAG + 1,
                              sidx(0, 2 * hv + 1) - 1)
                    pro += [(due, f) for f in v_group(jt, hv)]
            pro.sort(key=lambda t: t[0])
            proj_q.extend(pro)

            # ---- attention over q super-blocks ----
            for jq in range(NQ):
                n_ik = 4 * jq + 4
                # within the ACT-bound final jq, pending AVs carry ACROSS
                # quarter boundaries (qt<3 norms queue no outproj work, so
                # deferring them is free) — the exp chain then never
                # stalls on a boundary drain
                carry = (jq == NQ - 1)
                pending = deque()
                if jq + 1 < NQ:
                    proj_q.extend(seg_items(jq + 1))

                for qt in range(NQ):   # quarter = head pair (2qt, 2qt+1)
                    # y PSUM: bank b holds q-chunks 2b, 2b+1 as four
                    # 65-wide slots (2 qc x 2 heads, 65th col = denom).
                    # PSUM zero-region rule: one open accumulation group
                    # per bank — open at the bank's first AV, close at its
                    # last (precomputed from emission order), so bank 0
                    # (qc 0,1) closes early and can normalize while bank 1
                    # still accumulates.
                    # carry mode: lazy ybanks (alloc at first AV, when the
                    # previous quarter's banks have closed)
                    ybank = [] if carry else [
                        ps_y.tile([P, QB], f32, name="yb")
                        [:, 0:4 * VE].rearrange("p (s e) -> p s e", e=VE)
                        for _ in range(2)]
                    rec = rec_pool.tile([P, 8], f32, name="rec")
                    if qt == 0:
                        ysb = ysb_pool.tile([P, NQ, DL], bf16, name="ysb")

                    avs = []   # AV emission order: (ik, parity, qc)
                    for ik_ in range(n_ik):
                        pd_ = ik_ - 4 * jq
                        for pr_ in range(2):
                            for qc_ in range(max(0, pd_), 4):
                                avs.append((ik_, pr_, qc_))
                    first_b = {}
                    last_b = {}
                    for i_, key in enumerate(avs):
                        b_ = key[2] // 2
                        first_b.setdefault(b_, i_)
                        last_b[b_] = i_
                    av_flags = {}
                    for i_, key in enumerate(avs):
                        b_ = key[2] // 2
                        av_flags[key] = (first_b[b_] == i_, last_b[b_] == i_)
                    # after how many drained AV-steps is bank b closed?
                    bank_done = {b_: next(
                        i_ for i_, key in enumerate(avs) if
                        av_flags[key][1] and key[2] // 2 == b_)
                        for b_ in (0, 1)}

                    def emit_norm(qc, jq=jq, qt=qt, ybank=ybank, rec=rec,
                                  ysb=ysb):
                        b, s0 = qc // 2, (qc % 2) * 2
                        nc.vector.reciprocal(
                            rec[:, qc * 2:(qc + 1) * 2],
                            ybank[b][:, s0:s0 + 2, HD])
                        for pr in range(2):
                            h = 2 * qt + pr
                            nc.vector.tensor_scalar_mul(
                                ysb[:, qc, h * HD:(h + 1) * HD],
                                ybank[b][:, s0 + pr, 0:HD],
                                rec[:, qc * 2 + pr:qc * 2 + pr + 1])
                        if qt == NQ - 1:
                            # qc fully normalized: transpose + queue outproj
                            if jq == NQ - 1:
                                # final jq: PE-transpose (PE is starving
                                # here anyway) skips the ~3.7us XBAR DMA
                                # chain on the critical tail
                                tps = ps_s.tile([P, 2, QB], f32,
                                                name="sT_ps")
                                tpb = tps.bitcast(bf16)
                                for c in range(FC):
                                    nc.tensor.matmul(
                                        tpb[:, 0, c * P:(c + 1) * P],
                                        lhsT=ysb[:, qc, c * P:(c + 1) * P],
                                        rhs=ident_sb, is_transpose=True,
                                        start=(c == 0), stop=(c == FC - 1))
                                # qc<2: ACT is still chewing the last
                                # exps, copy on DVE; qc>=2: ACT is free
                                cpy = (nc.vector.tensor_copy if qc < 2
                                       else nc.scalar.copy)
                                cpy(yT_sb[:, :, jq * QB + qc * P:
                                          jq * QB + (qc + 1) * P],
                                    tpb[:, 0, 0:DL].rearrange(
                                        "p (c w) -> p c w", c=FC))
                                sh = {}
                                for nd in range(ND):
                                    opt_q.extend(outproj_group(
                                        jq * NQ + qc, nd, sh, False))
                            else:
                                nc.sync.dma_start_transpose(
                                    yT_sb[:, :, jq * QB + qc * P:
                                          jq * QB + (qc + 1) * P],
                                    ysb[:, qc, :])
                                sh = {}
                                for nd in range(ND):
                                    opt_q.extend(outproj_group(
                                        jq * NQ + qc, nd, sh, True))

                    normed = [0]
                    drained = [0]   # AV *instructions* drained
                    # ACT-bound final jq: shallower exp->AV pipeline keeps
                    # AV work in-loop as filler instead of piling it into
                    # the post-loop drain (exp latency is still covered)
                    lag = LAGF if jq == NQ - 1 else LAG

                    def try_norm(normed=normed, drained=drained,
                                 bank_done=bank_done, emit_norm=emit_norm):
                        while (normed[0] < NQ
                               and drained[0] > bank_done[normed[0] // 2]):
                            emit_norm(normed[0])
                            normed[0] += 1

                    for ik in range(n_ik):
                        pd = ik - 4 * jq
                        c0 = max(0, pd * P)
                        g = sidx(jq, qt, ik)
                        while proj_q and proj_q[0][0] <= g:
                            proj_q.popleft()[1]()
                        sprinkle(g)
                        while len(pending) > lag:
                            pending.popleft()()
                        try_norm()
                        ps = ps_s.tile([P, 2, QB], f32, name="sT_ps")
                        for pr in range(2):
                            nc.tensor.matmul(
                                ps[:, pr, c0:QB],
                                lhsT=kT_sb[pr * HD:(pr + 1) * HD, qt,
                                           ik * P:(ik + 1) * P],
                                rhs=qT_sb[pr * HD:(pr + 1) * HD, qt,
                                          jq * QB + c0:(jq + 1) * QB],
                                start=True, stop=True)
                        sT = sT_pool.tile([P, 2, QB], bf16)
                        nc.scalar.activation(
                            out=sT[:, :, c0:QB], in_=ps[:, :, c0:QB],
                            func=Exp, scale=0.125)
                        if pd >= 0:
                            mb = bass.AP(
                                tensor=mask_sb.tensor, offset=mask_sb.offset,
                                ap=[mask_sb.ap[0], [0, 2], mask_sb.ap[1]])
                            nc.vector.tensor_mul(
                                sT[:, :, c0:c0 + P],
                                sT[:, :, c0:c0 + P], mb)

                        def av(ik=ik, qt=qt, sT=sT, pd=pd, jq=jq,
                               ybank=ybank, av_flags=av_flags,
                               drained=drained, tn=try_norm):
                            if not ybank:
                                ybank.extend(
                                    ps_y.tile([P, QB], f32, name="yb")
                                    [:, 0:4 * VE].rearrange(
                                        "p (s e) -> p s e", e=VE)
                                    for _ in range(2))
                            for pr in range(2):
                                for qc in range(max(0, pd), 4):
                                    st, sp = av_flags[(ik, pr, qc)]
                                    nc.tensor.matmul(
                                        ybank[qc // 2][:, (qc % 2) * 2 + pr,
                                                       :],
                                        lhsT=sT[:, pr, qc * P:(qc + 1) * P],
                                        rhs=v_sb[:, ik, 2 * qt + pr, :],
                                        start=st, stop=sp)
                            drained[0] += (4 - max(0, pd)) * 2
                            tn()   # norms fire as soon as banks close
                        pending.append(av)
                    if carry and qt < NQ - 1:
                        continue   # pending carries into the next quarter
                    # quarter ends: anything due inside it must be emitted
                    # before the remaining AVs (which may consume it)
                    g_end = sidx(jq, qt) + n_ik
                    while proj_q and proj_q[0][0] <= g_end:
                        proj_q.popleft()[1]()
                    while pending:
                        pending.popleft()()

            while proj_q:
                proj_q.popleft()[1]()
            while opt_q:
                opt_q.popleft()()

    nc.finalize()
    return nc


def _mchunk(W):
    """[D, DL] -> [FC, P, KC, P] m-major chunks: Wh[m,p,c,w] = W[c*P+p, m*P+w]."""
    return np.ascontiguousarray(
        W.reshape(KC, P, FC, P).transpose(2, 1, 0, 3))


def _in_maps(x, Wq, bq, Wk, Wv, Wp):
    import ml_dtypes
    bf16 = ml_dtypes.bfloat16
    f8 = ml_dtypes.float8_e4m3fn
    mask = np.triu(np.ones((P, P), dtype=np.float32)).astype(bf16)
    maps = []
    for c in range(8):
        b, hg = divmod(c, 2)
        sl = slice(hg * DL, (hg + 1) * DL)
        xtb = np.ascontiguousarray(x[b].T)
        xc = xtb[FPC * P:(FPC + 1) * P]          # [128, T] chunk rows
        wc = Wv[FPC * P:(FPC + 1) * P, sl]       # [128, DL]
        x8 = xc.reshape(2, P // 2, T).transpose(1, 0, 2)   # k = i*64 + p
        w8 = wc.reshape(2, P // 2, DL).transpose(1, 0, 2)
        maps.append({
            "xt": xtb.astype(bf16),
            "xt8": np.ascontiguousarray(x8).astype(f8),
            "wv8": np.ascontiguousarray(w8).astype(f8),
            "wq": _mchunk(Wq[:, sl]).astype(bf16),
            "wk": _mchunk(Wk[:, sl]).astype(bf16),
            "wv": np.ascontiguousarray(Wv[:, sl]).astype(bf16),
            "wp": np.ascontiguousarray(Wp[sl, :]).astype(bf16),
            "bq": np.ascontiguousarray(bq[sl]),
            "mask": mask,
            "ident": np.eye(P, dtype=np.float32).astype(bf16),
        })
    return maps


def kernel(x, Wq, bq, Wk, bk, Wv, bv, Wp, bp):
    from concourse.bass_utils import run_bass_kernel_spmd

    if "nc" not in _CACHE:
        _CACHE["nc"] = _build()
    nc = _CACHE["nc"]

    x = np.asarray(x, np.float32)
    Wq, bq, Wk, bk, Wv, bv, Wp = [
        np.asarray(a, np.float32) for a in (Wq, bq, Wk, bk, Wv, bv, Wp)]
    bp = np.asarray(bp, np.float32)

    in_maps = _in_maps(x, Wq, bq, Wk, Wv, Wp)
    _CACHE["in_maps"] = in_maps

    # bk cancels in softmax; bv reaches the output as the constant bv@Wp
    obias = bv @ Wp + bp
    res = run_bass_kernel_spmd(nc, in_maps, list(range(8))).results
    out = np.empty((B, T, D), dtype=np.float32)
    for b in range(B):
        out[b] = (res[2 * b]["out"].astype(np.float32)
                  + res[2 * b + 1]["out"].astype(np.float32) + obias)
    return out



# revision 109
# speedup vs baseline: 1.0026x; 1.0026x over previous
"""Causal multi-head attention on 8 Trainium2 NeuronCores.

Problem: x[4, 2048, 1024], 16 heads of 64; q/k/v = x@W* + b*, causal
softmax attention, out = y@Wp + bp.

Sharding: core c handles batch b = c//2 and head-group hg = c%2
(8 heads = 512 feature columns of Wq/Wk/Wv, 512 rows of Wp).  Each core
computes a full [2048, 1024] partial of the output projection for its
batch; the host sums the two partials per batch and adds bp.

Per-core dataflow (bf16 matmul inputs, fp32 PSUM accumulation):
  * x arrives in DRAM already TRANSPOSED by the host (xt [D, T]) so xT
    streams in as cheap flat DMAs interleaved with the weights in
    first-use order; wq/wk are m-chunked on the host so each 256KB
    first-use piece is one contiguous transfer.  No device transposes
    of x at all.  bk/bv never reach the device: q.(k+bk) shifts every
    score in a softmax row equally (cancels exactly), and since softmax
    rows sum to 1, bv contributes the constant bv@Wp folded into the
    host-side output bias.
  * a PE p-state warmup (dummy matmuls on memset scratch) runs during
    the cold DMA window so the 3us clock ramp completes before real
    work starts.
  * qT/kT [head-pair-on-partitions, t] and v [t, heads*65] (65th column
    of each head's v block is ones so softmax denominators fall out of
    the same AV matmuls).
  * attention runs in QUARTER-passes of one head pair (hc): per
    (quarter, k-block 128, q-super-block 512) BOTH heads' score tiles
    land in one [128, 2, 512] two-bank PSUM tile and share ONE exp
    instruction (3D AP; halves the ACT per-instruction overhead, the
    second engine bottleneck).  Triangular mask on the diagonal blocks
    is one stride-0-broadcast DVE multiply for both heads.
  * AV is REORIENTED: per q-chunk of 128, y[q, 65] += sT_chunk.T @ v_ext
    (lhsT = sT chunk) streams only 65 output rows per (q128, k128) pair
    instead of 512 — half the PE time of the yT-oriented form.  The 4
    slots (2 qc x 2 heads) of a PSUM y-bank share ONE accumulation
    group (hardware zero-regions are whole banks), opened by the bank's
    first AV and closed by its last, so early q-chunks normalize while
    later k-blocks still accumulate the other bank.
  * softmax normalization is a per-partition scalar multiply (DVE) while
    copying y PSUM->SBUF bf16; y is DMA-transposed to yT for the output
    projection, whose result is staged bf16 and DMA'd to DRAM (the host
    sums the two partials per batch in fp32).  For the FINAL jq the yT
    transpose runs on the PE instead (it is starving there anyway),
    skipping the ~3.7us XBAR DMA chain on the critical tail, and each
    nd half DMAs on its own so the last transfer is small; earlier jqs
    fuse both nd halves into one whole-row DMA (half the HWDGE issues).
  * the PE instruction stream is software-pipelined by an emission
    scheduler: projection / output-projection matmuls carry deadline
    tags (first-use attention step) and are paced into the exp-limited
    attention steps by a most-binding-prefix rate, with out-projection
    work deferred into the late, exp-bound region.  Work-group PSUM
    tiles alternate across two pools so group copies never serialize.
  * within the ACT-bound final jq, pending AVs carry ACROSS quarter
    boundaries (qt<3 norms queue no outproj work so deferring them is
    free; ybanks allocate lazily at each quarter's first AV) — the exp
    chain never stalls waiting for an end-of-quarter drain.
"""
import numpy as np
from collections import deque

B, T, D = 4, 2048, 1024
NH, HD = 16, 64
NHL = 8            # heads per core
DL = NHL * HD      # 512: local qkv feature width
P = 128
QB = 512           # q super-block (columns of sT tiles)
NQ = T // QB       # 4
NKT = T // P       # 16 k blocks
KC = D // P        # 8 contraction chunks over model dim
FC = DL // P       # 4 chunks over local feature dim
DB = 512           # out-projection column block
ND = D // DB       # 2
VE = HD + 1        # 65: v block width incl ones column
LAG = 7            # exp->AV software pipeline depth (in (ik,h) steps)
LAGF = 8           # pipeline depth for the final jq (8 = same as LAG)
OPTH = 28          # outproj pacing horizon (steps past TOT)
BNDI = 3           # base index where the late feed region starts
LATE = 1.212       # outproj feed factor in the exp-bound late region
EARLY = 0.3        # outproj feed factor before the late region
FQB = 0.0          # extra per-step outproj top-up in the final quarter
FPC = 7            # v-proj contraction chunk that runs fp8-DoubleRow

_CACHE = {}


def _build():
    import concourse.bass as bass
    from concourse import bacc
    import concourse.mybir as mybir
    import concourse.tile as tile

    f32 = mybir.dt.float32
    bf16 = mybir.dt.bfloat16
    Exp = mybir.ActivationFunctionType.Exp

    f8 = mybir.dt.float8e4
    DR = mybir.MatmulPerfMode.DoubleRow

    nc = bacc.Bacc(None)
    xt_d = nc.dram_tensor("xt", [D, T], bf16, kind="ExternalInput")
    # ONE contraction chunk (FPC) of the v projection runs fp8-DoubleRow
    # (half-rate rows): costs 1.4e-2 rel err vs the 2e-2 gate. Operands
    # arrive host-packed [K/2, 2, N] with k = i*64 + p on both sides.
    xt8_d = nc.dram_tensor("xt8", [P // 2, 2, T], f8, kind="ExternalInput")
    wv8_d = nc.dram_tensor("wv8", [P // 2, 2, DL], f8, kind="ExternalInput")
    # wq/wk arrive m-chunked from the host ([m, p, c, w] with w the
    # 128-col lhsT slice) so each 256KB chunk is one contiguous DMA
    wq_d = nc.dram_tensor("wq", [FC, P, KC, P], bf16, kind="ExternalInput")
    wk_d = nc.dram_tensor("wk", [FC, P, KC, P], bf16, kind="ExternalInput")
    wv_d = nc.dram_tensor("wv", [D, DL], bf16, kind="ExternalInput")
    wp_d = nc.dram_tensor("wp", [DL, D], bf16, kind="ExternalInput")
    bq_d = nc.dram_tensor("bq", [DL], f32, kind="ExternalInput")
    mask_d = nc.dram_tensor("mask", [P, P], bf16, kind="ExternalInput")
    ident_d = nc.dram_tensor("ident", [P, P], bf16, kind="ExternalInput")
    out_d = nc.dram_tensor("out", [T, D], bf16, kind="ExternalOutput")
    # bk and bv never reach the device: q.(k+bk) shifts every score in a
    # softmax row by the same q.bk, so bk cancels exactly; and since
    # softmax rows sum to 1, v's bias adds a constant bv to y, so
    # bv@Wp folds into the host-side output bias.

    with tile.TileContext(nc) as tc:
        with (
            tc.tile_pool(name="const", bufs=1) as const,
            tc.tile_pool(name="big", bufs=1) as big,
            tc.tile_pool(name="sT", bufs=23) as sT_pool,
            tc.tile_pool(name="ysb", bufs=3) as ysb_pool,
            tc.tile_pool(name="rec", bufs=4) as rec_pool,
            tc.tile_pool(name="ostage", bufs=6) as stage_pool,
            tc.tile_pool(name="ps_s", bufs=2, space="PSUM") as ps_s,
            tc.tile_pool(name="ps_y", bufs=3, space="PSUM") as ps_y,
            tc.tile_pool(name="ps_w", bufs=1, space="PSUM") as ps_w,
        ):
            # ---- persistent SBUF ----
            xT_sb = big.tile([P, KC, T], bf16)
            wq_sb = big.tile([P, FC, KC, P], bf16)
            wk_sb = big.tile([P, FC, KC, P], bf16)
            wv_sb = big.tile([P, KC, DL], bf16)
            wp_sb = big.tile([P, FC, D], bf16)
            qT_sb = big.tile([P, FC, T], bf16)
            kT_sb = big.tile([P, FC, T], bf16)
            v_sb = big.tile([P, NKT, NHL, VE], bf16)
            yT_sb = big.tile([P, FC, T], bf16)
            xt8_sb = big.tile([P // 2, 2, T], f8)
            wv8_sb = big.tile([P // 2, 2, DL], f8)
            mask_sb = const.tile([P, P], bf16)
            ident_sb = const.tile([P, P], bf16)
            bq_sb = const.tile([P, FC], f32)

            # ---- prologue DMAs (SP HWDGE queue; order = issue order,
            # sequenced to match first-use times of the PE stream).  x
            # arrives already transposed from the host, so xT streams in
            # as flat DMAs interleaved with the weights by first use.
            # wq/wk are m-chunked on the host so each first-use piece is
            # a short contiguous 256KB transfer. ----
            xt_r = xt_d.ap().rearrange("(c p) t -> p c t", p=P)
            H = 2 * P   # wv half-width: 256 cols = 512B rows, full DMA bw
            nc.sync.dma_start(wk_sb[:, 0], wk_d.ap()[0])
            for cp in range(2):
                nc.sync.dma_start(
                    xT_sb[:, 2 * cp:2 * cp + 2, 0:QB],
                    xt_r[:, 2 * cp:2 * cp + 2, 0:QB])
            nc.sync.dma_start(wk_sb[:, 1], wk_d.ap()[1])
            nc.sync.dma_start(xT_sb[:, 4:8, 0:QB], xt_r[:, 4:8, 0:QB])
            nc.sync.dma_start(wq_sb[:, 0], wq_d.ap()[0])
            wv_r = wv_d.ap().rearrange("(c p) m -> p c m", p=P)
            nc.sync.dma_start(wv_sb[:, :, 0:H], wv_r[:, :, 0:H])
            nc.sync.dma_start(bq_sb, bq_d.ap().rearrange("(c p) -> p c", p=P))
            nc.sync.dma_start(mask_sb, mask_d.ap())
            nc.sync.dma_start(wv8_sb, wv8_d.ap())
            nc.sync.dma_start(xt8_sb, xt8_d.ap())
            nc.sync.dma_start(wq_sb[:, 1], wq_d.ap()[1])
            nc.sync.dma_start(wk_sb[:, 2], wk_d.ap()[2])
            nc.sync.dma_start(wq_sb[:, 2], wq_d.ap()[2])
            nc.sync.dma_start(wv_sb[:, :, H:DL], wv_r[:, :, H:DL])
            nc.sync.dma_start(wk_sb[:, 3], wk_d.ap()[3])
            nc.sync.dma_start(wq_sb[:, 3], wq_d.ap()[3])
            for seg in range(1, NQ):
                nc.sync.dma_start(
                    xT_sb[:, :, seg * QB:(seg + 1) * QB],
                    xt_r[:, :, seg * QB:(seg + 1) * QB])
            nc.sync.dma_start(
                wp_sb, wp_d.ap().rearrange("(c p) m -> p c m", p=P))
            nc.sync.dma_start(ident_sb, ident_d.ap())

            # ---- PE p-state warmup: dummy matmuls on a memset scratch
            # keep the PE busy from ~1.5us so the 3us continuous-busy
            # ramp completes early; more warmups are interleaved into the
            # first projection group below wherever the cold DMA chain
            # would otherwise stall (and reset the ramp) ----
            wup_sb = const.tile([P, QB], bf16)
            nc.vector.memset(wup_sb, 0.0)
            nc.vector.memset(v_sb[:, :, :, HD], 1.0)   # ones columns
            wups = ps_w.tile([P, QB], f32, name="psw")

            def wup(n):
                for r in range(n):
                    nc.tensor.matmul(wups, lhsT=wup_sb[:, 0:P],
                                     rhs=wup_sb, start=True, stop=True)
            wup(7)

            # ---- work-item machinery (each item emits ~1 PE matmul) ----
            def group_items(n_mm, emit_mm, emit_tail):
                cell = {}
                items = []
                for i in range(n_mm):
                    def it(i=i):
                        emit_mm(i, cell)
                        if i == n_mm - 1:
                            emit_tail(cell)
                    items.append(it)
                return items

            # work-group PSUM: alternate between the ps_w bank and the
            # (mostly idle between y-uses) ps_y banks so pipelined groups
            # never serialize on one bank's copy latency
            rot = [0]

            def mkps_rot():
                rot[0] ^= 1
                return (ps_w.tile([P, QB], f32, name="psw") if rot[0]
                        else ps_y.tile([P, QB], f32, name="yb"))

            def v_group(jt, hv):
                """Half of the v projection for t-block jt: feature columns
                [hv*256, (hv+1)*256) = heads 4hv..4hv+3."""
                c0, c1 = hv * H, (hv + 1) * H

                def mm(i, cell):
                    if i == 0:
                        cell["ps"] = mkps_rot()
                    if i == FPC:
                        nc.tensor.matmul(
                            cell["ps"][:, c0:c1],
                            lhsT=xt8_sb[:, :, jt * P:(jt + 1) * P],
                            rhs=wv8_sb[:, :, c0:c1],
                            start=(i == 0), stop=(i == KC - 1),
                            perf_mode=DR)
                    else:
                        nc.tensor.matmul(
                            cell["ps"][:, c0:c1],
                            lhsT=xT_sb[:, i, jt * P:(jt + 1) * P],
                            rhs=wv_sb[:, i, c0:c1],
                            start=(i == 0), stop=(i == KC - 1))

                def tail(cell):
                    nc.vector.tensor_copy(
                        v_sb[:, jt, 4 * hv:4 * hv + 4, 0:HD],
                        cell["ps"][:, c0:c1].rearrange(
                            "p (h e) -> p h e", h=4))
                return group_items(KC, mm, tail)

            def qk_group(w_sb, b_sb, dst, m, seg, mkps=None):
                def mm(i, cell):
                    if i == 0:
                        cell["ps"] = mkps() if mkps else mkps_rot()
                    nc.tensor.matmul(
                        cell["ps"],
                        lhsT=w_sb[:, m, i, :],
                        rhs=xT_sb[:, i, seg * QB:(seg + 1) * QB],
                        start=(i == 0), stop=(i == KC - 1))

                def tail(cell):
                    if b_sb is None:
                        nc.vector.tensor_copy(
                            dst[:, m, seg * QB:(seg + 1) * QB], cell["ps"])
                    else:
                        nc.vector.tensor_scalar_add(
                            dst[:, m, seg * QB:(seg + 1) * QB], cell["ps"],
                            b_sb[:, m:m + 1])
                return group_items(KC, mm, tail)

            def outproj_group(tb, nd, shared, fuse, h=None, mkps=None):
                """fuse: both nd halves land in one shared [P, D] stage
                tile and ONE whole-row DMA fires at nd=1's tail (halves
                the HWDGE issue load); unfused (final jq) each nd half
                DMAs on its own so the last transfer is small; h splits
                an unfused half again (very last group only)."""
                c0 = nd * DB + (0 if h is None else h * (DB // 2))
                w = DB if h is None else DB // 2

                def mm(c, cell):
                    if c == 0:
                        cell["ps"] = (mkps or mkps_rot)()
                    nc.tensor.matmul(
                        cell["ps"][:, 0:w],
                        lhsT=yT_sb[:, c, tb * P:(tb + 1) * P],
                        rhs=wp_sb[:, c, c0:c0 + w],
                        start=(c == 0), stop=(c == FC - 1))

                def tail(cell):
                    if "st" not in shared:
                        shared["st"] = stage_pool.tile(
                            [P, D], bf16, name="ostage")
                    ot = shared["st"]
                    nc.vector.tensor_copy(
                        ot[:, c0:c0 + w], cell["ps"][:, 0:w])
                    if fuse and nd == ND - 1:
                        nc.sync.dma_start(
                            out_d.ap()[tb * P:(tb + 1) * P, :], ot)
                    elif not fuse:
                        nc.sync.dma_start(
                            out_d.ap()[tb * P:(tb + 1) * P, c0:c0 + w],
                            ot[:, c0:c0 + w])
                return group_items(FC, mm, tail)

            def seg_groups(seg):
                gs = []
                gs += qk_group(wk_sb, None, kT_sb, 0, seg)
                gs += qk_group(wk_sb, None, kT_sb, 1, seg)
                gs += qk_group(wq_sb, bq_sb, qT_sb, 0, seg)
                gs += qk_group(wq_sb, bq_sb, qT_sb, 1, seg)
                for jt in range(seg * NQ, seg * NQ + NQ):
                    gs += v_group(jt)
                gs += qk_group(wk_sb, None, kT_sb, 2, seg)
                gs += qk_group(wk_sb, None, kT_sb, 3, seg)
                gs += qk_group(wq_sb, bq_sb, qT_sb, 2, seg)
                gs += qk_group(wq_sb, bq_sb, qT_sb, 3, seg)
                return gs

            # global step index: one step per (jq, quarter=head-pair, ik)
            base = [0]
            for j in range(NQ):
                base.append(base[-1] + 4 * (4 * j + 4))
            TOT = base[NQ]   # 160

            def sidx(jq, qt, ik=0):
                return base[jq] + qt * (4 * jq + 4) + ik

            proj_q = deque()   # items: (due_step, fn) in due order
            opt_q = deque()    # out-projection items (no deadline)
            pace = [0.0, 0.0]

            def seg_items(seg):
                """Projection work for seg, tagged with first-use steps.
                kT/qT chunk m is first read by quarter m (hc == m)."""
                its = []
                for m in range(FC):
                    its += [(sidx(seg, m), f) for f in
                            qk_group(wk_sb, None, kT_sb, m, seg)]
                    its += [(sidx(seg, m), f) for f in
                            qk_group(wq_sb, bq_sb, qT_sb, m, seg)]
                    if m < 2:
                        for jt in (seg * NQ + 2 * m, seg * NQ + 2 * m + 1):
                            for hv in range(2):
                                due = min(sidx(seg, 2 * hv, jt) + LAG + 1,
                                          sidx(seg, 2 * hv + 1) - 1)
                                its += [(due, f) for f in v_group(jt, hv)]
                its.sort(key=lambda t: t[0])
                return its

            def sprinkle(g):
                """Pace proj work by most-binding-prefix rate; spread opt
                work over all remaining steps."""
                if proj_q:
                    r = max((i + 1) / max(1, due - g)
                            for i, (due, _) in enumerate(
                                list(proj_q)[:24]))
                    pace[0] += min(r, 2.0)
                    while pace[0] >= 1.0 and proj_q:
                        proj_q.popleft()[1]()
                        pace[0] -= 1.0
                if opt_q:
                    # defer outproj into the late (exp-bound) region and
                    # hold a reserve for the ACT-bound final quarter
                    r = len(opt_q) / max(1.0, TOT + OPTH - g)
                    f = EARLY if g < base[BNDI] else LATE
                    pace[1] += r * f
                    if g >= base[NQ - 1] + 3 * (4 * (NQ - 1) + 4):
                        # final quarter: steps are exp-clocked ~2 items
                        # slower than PE; rate-pacing underfeeds, so top up
                        pace[1] += FQB
                    while pace[1] >= 1.0 and opt_q:
                        opt_q.popleft()()
                        pace[1] -= 1.0

            # ---- prologue: just enough of seg 0 to start attention;
            # groups run on the idle sT psum banks for deep pipelining ----
            def ps_proto():
                return ps_s.tile([P, 2, QB], f32, name="sT_ps")[:, 0, :]

            # first group: warmup filler between i-chunks where the cold
            # DMA chain (xt45/xt67 still in flight) would stall the PE
            for f in qk_group(wk_sb, None, kT_sb, 0, 0, ps_proto):
                f()
            for f in qk_group(wk_sb, None, kT_sb, 1, 0, ps_proto):
                f()
            for f in qk_group(wq_sb, bq_sb, qT_sb, 0, 0, ps_proto):
                f()
            pro = []
            pro += [(sidx(0, 1), f) for f in
                    qk_group(wq_sb, bq_sb, qT_sb, 1, 0, ps_proto)]
            for m in (2, 3):
                pro += [(sidx(0, m), f) for f in
                        qk_group(wk_sb, None, kT_sb, m, 0, ps_proto)]
                pro += [(sidx(0, m), f) for f in
                        qk_group(wq_sb, bq_sb, qT_sb, m, 0, ps_proto)]
            for jt in range(NQ):
                for hv in range(2):
                    due = min(sidx(0, 2 * hv, jt) + LAG + 1,
                              sidx(0, 2 * hv + 1) - 1)
                    pro += [(due, f) for f in v_group(jt, hv)]
            pro.sort(key=lambda t: t[0])
            proj_q.extend(pro)

            # ---- attention over q super-blocks ----
            for jq in range(NQ):
                n_ik = 4 * jq + 4
                # within the ACT-bound final jq, pending AVs carry ACROSS
                # quarter boundaries (qt<3 norms queue no outproj work, so
                # deferring them is free) — the exp chain then never
                # stalls on a boundary drain
                carry = (jq == NQ - 1)
                pending = deque()
                if jq + 1 < NQ:
                    proj_q.extend(seg_items(jq + 1))

                for qt in range(NQ):   # quarter = head pair (2qt, 2qt+1)
                    # y PSUM: bank b holds q-chunks 2b, 2b+1 as four
                    # 65-wide slots (2 qc x 2 heads, 65th col = denom).
                    # PSUM zero-region rule: one open accumulation group
                    # per bank — open at the bank's first AV, close at its
                    # last (precomputed from emission order), so bank 0
                    # (qc 0,1) closes early and can normalize while bank 1
                    # still accumulates.
                    # carry mode: lazy ybanks (alloc at first AV, when the
                    # previous quarter's banks have closed)
                    ybank = [] if carry else [
                        ps_y.tile([P, QB], f32, name="yb")
                        [:, 0:4 * VE].rearrange("p (s e) -> p s e", e=VE)
                        for _ in range(2)]
                    rec = rec_pool.tile([P, 8], f32, name="rec")
                    if qt == 0:
                        ysb = ysb_pool.tile([P, NQ, DL], bf16, name="ysb")

                    avs = []   # AV emission order: (ik, parity, qc)
                    for ik_ in range(n_ik):
                        pd_ = ik_ - 4 * jq
                        for pr_ in range(2):
                            for qc_ in range(max(0, pd_), 4):
                                avs.append((ik_, pr_, qc_))
                    first_b = {}
                    last_b = {}
                    for i_, key in enumerate(avs):
                        b_ = key[2] // 2
                        first_b.setdefault(b_, i_)
                        last_b[b_] = i_
                    av_flags = {}
                    for i_, key in enumerate(avs):
                        b_ = key[2] // 2
                        av_flags[key] = (first_b[b_] == i_, last_b[b_] == i_)
                    # after how many drained AV-steps is bank b closed?
                    bank_done = {b_: next(
                        i_ for i_, key in enumerate(avs) if
                        av_flags[key][1] and key[2] // 2 == b_)
                        for b_ in (0, 1)}

                    def emit_norm(qc, jq=jq, qt=qt, ybank=ybank, rec=rec,
                                  ysb=ysb):
                        b, s0 = qc // 2, (qc % 2) * 2
                        nc.vector.reciprocal(
                            rec[:, qc * 2:(qc + 1) * 2],
                            ybank[b][:, s0:s0 + 2, HD])
                        for pr in range(2):
                            h = 2 * qt + pr
                            nc.vector.tensor_scalar_mul(
                                ysb[:, qc, h * HD:(h + 1) * HD],
                                ybank[b][:, s0 + pr, 0:HD],
                                rec[:, qc * 2 + pr:qc * 2 + pr + 1])
                        if qt == NQ - 1:
                            # qc fully normalized: transpose + queue outproj
                            if jq == NQ - 1:
                                # final jq: PE-transpose (PE is starving
                                # here anyway) skips the ~3.7us XBAR DMA
                                # chain on the critical tail
                                tps = ps_s.tile([P, 2, QB], f32,
                                                name="sT_ps")
                                tpb = tps.bitcast(bf16)
                                for c in range(FC):
                                    nc.tensor.matmul(
                                        tpb[:, 0, c * P:(c + 1) * P],
                                        lhsT=ysb[:, qc, c * P:(c + 1) * P],
                                        rhs=ident_sb, is_transpose=True,
                                        start=(c == 0), stop=(c == FC - 1))
                                # qc<2: ACT is still chewing the last
                                # exps, copy on DVE; qc>=2: ACT is free
                                cpy = (nc.vector.tensor_copy if qc < 2
                                       else nc.scalar.copy)
                                cpy(yT_sb[:, :, jq * QB + qc * P:
                                          jq * QB + (qc + 1) * P],
                                    tpb[:, 0, 0:DL].rearrange(
                                        "p (c w) -> p c w", c=FC))
                                sh = {}
                                for nd in range(ND):
                                    opt_q.extend(outproj_group(
                                        jq * NQ + qc, nd, sh, False))
                            else:
                                nc.sync.dma_start_transpose(
                                    yT_sb[:, :, jq * QB + qc * P:
                                          jq * QB + (qc + 1) * P],
                                    ysb[:, qc, :])
                                sh = {}
                                for nd in range(ND):
                                    opt_q.extend(outproj_group(
                                        jq * NQ + qc, nd, sh, True))

                    normed = [0]
                    drained = [0]   # AV *instructions* drained
                    # ACT-bound final jq: shallower exp->AV pipeline keeps
                    # AV work in-loop as filler instead of piling it into
                    # the post-loop drain (exp latency is still covered)
                    lag = LAGF if jq == NQ - 1 else LAG

                    def try_norm(normed=normed, drained=drained,
                                 bank_done=bank_done, emit_norm=emit_norm):
                        while (normed[0] < NQ
                               and drained[0] > bank_done[normed[0] // 2]):
                            emit_norm(normed[0])
                            normed[0] += 1

                    for ik in range(n_ik):
                        pd = ik - 4 * jq
                        c0 = max(0, pd * P)
                        g = sidx(jq, qt, ik)
                        while proj_q and proj_q[0][0] <= g:
                            proj_q.popleft()[1]()
                        sprinkle(g)
                        while len(pending) > lag:
                            pending.popleft()()
                        try_norm()
                        ps = ps_s.tile([P, 2, QB], f32, name="sT_ps")
                        for pr in range(2):
                            nc.tensor.matmul(
                                ps[:, pr, c0:QB],
                                lhsT=kT_sb[pr * HD:(pr + 1) * HD, qt,
                                           ik * P:(ik + 1) * P],
                                rhs=qT_sb[pr * HD:(pr + 1) * HD, qt,
                                          jq * QB + c0:(jq + 1) * QB],
                                start=True, stop=True)
                        sT = sT_pool.tile([P, 2, QB], bf16)
                        nc.scalar.activation(
                            out=sT[:, :, c0:QB], in_=ps[:, :, c0:QB],
                            func=Exp, scale=0.125)
                        if pd >= 0:
                            mb = bass.AP(
                                tensor=mask_sb.tensor, offset=mask_sb.offset,
                                ap=[mask_sb.ap[0], [0, 2], mask_sb.ap[1]])
                            nc.vector.tensor_mul(
                                sT[:, :, c0:c0 + P],
                                sT[:, :, c0:c0 + P], mb)

                        def av(ik=ik, qt=qt, sT=sT, pd=pd, jq=jq,
                               ybank=ybank, av_flags=av_flags,
                               drained=drained, tn=try_norm):
                            if not ybank:
                                ybank.extend(
                                    ps_y.tile([P, QB], f32, name="yb")
                                    [:, 0:4 * VE].rearrange(
                                        "p (s e) -> p s e", e=VE)
                                    for _ in range(2))
                            for pr in range(2):
                                for qc in range(max(0, pd), 4):
                                    st, sp = av_flags[(ik, pr, qc)]
                                    nc.tensor.matmul(
                                        ybank[qc // 2][:, (qc % 2) * 2 + pr,
                                                       :],
                                        lhsT=sT[:, pr, qc * P:(qc + 1) * P],
                                        rhs=v_sb[:, ik, 2 * qt + pr, :],
                                        start=st, stop=sp)
                            drained[0] += (4 - max(0, pd)) * 2
                            tn()   # norms fire as soon as banks close
                        pending.append(av)
                    if carry and qt < NQ - 1:
                        continue   # pending carries into the next quarter
                    # quarter ends: anything due inside it must be emitted
                    # before the remaining AVs (which may consume it)
                    g_end = sidx(jq, qt) + n_ik
                    while proj_q and proj_q[0][0] <= g_end:
                        proj_q.popleft()[1]()
                    while pending:
                        pending.popleft()()

            while proj_q:
                proj_q.popleft()[1]()
            while opt_q:
                opt_q.popleft()()

    nc.finalize()
    return nc


def _mchunk(W):
    """[D, DL] -> [FC, P, KC, P] m-major chunks: Wh[m,p,c,w] = W[c*P+p, m*P+w]."""
    return np.ascontiguousarray(
        W.reshape(KC, P, FC, P).transpose(2, 1, 0, 3))


def _in_maps(x, Wq, bq, Wk, Wv, Wp):
    import ml_dtypes
    bf16 = ml_dtypes.bfloat16
    f8 = ml_dtypes.float8_e4m3fn
    mask = np.triu(np.ones((P, P), dtype=np.float32)).astype(bf16)
    maps = []
    for c in range(8):
        b, hg = divmod(c, 2)
        sl = slice(hg * DL, (hg + 1) * DL)
        xtb = np.ascontiguousarray(x[b].T)
        xc = xtb[FPC * P:(FPC + 1) * P]          # [128, T] chunk rows
        wc = Wv[FPC * P:(FPC + 1) * P, sl]       # [128, DL]
        x8 = xc.reshape(2, P // 2, T).transpose(1, 0, 2)   # k = i*64 + p
        w8 = wc.reshape(2, P // 2, DL).transpose(1, 0, 2)
        maps.append({
            "xt": xtb.astype(bf16),
            "xt8": np.ascontiguousarray(x8).astype(f8),
            "wv8": np.ascontiguousarray(w8).astype(f8),
            "wq": _mchunk(Wq[:, sl]).astype(bf16),
            "wk": _mchunk(Wk[:, sl]).astype(bf16),
            "wv": np.ascontiguousarray(Wv[:, sl]).astype(bf16),
            "wp": np.ascontiguousarray(Wp[sl, :]).astype(bf16),
            "bq": np.ascontiguousarray(bq[sl]),
            "mask": mask,
            "ident": np.eye(P, dtype=np.float32).astype(bf16),
        })
    return maps


def kernel(x, Wq, bq, Wk, bk, Wv, bv, Wp, bp):
    from concourse.bass_utils import run_bass_kernel_spmd

    if "nc" not in _CACHE:
        _CACHE["nc"] = _build()
    nc = _CACHE["nc"]

    x = np.asarray(x, np.float32)
    Wq, bq, Wk, bk, Wv, bv, Wp = [
        np.asarray(a, np.float32) for a in (Wq, bq, Wk, bk, Wv, bv, Wp)]
    bp = np.asarray(bp, np.float32)

    in_maps = _in_maps(x, Wq, bq, Wk, Wv, Wp)
    _CACHE["in_maps"] = in_maps

    # bk cancels in softmax; bv reaches the output as the constant bv@Wp
    obias = bv @ Wp + bp
    res = run_bass_kernel_spmd(nc, in_maps, list(range(8))).results
    out = np.empty((B, T, D), dtype=np.float32)
    for b in range(B):
        out[b] = (res[2 * b]["out"].astype(np.float32)
                  + res[2 * b + 1]["out"].astype(np.float32) + obias)
    return out

